# revision 7
# baseline (speedup 1.0000x reference)
"""MLA-style Llama attention kernel for 8 Trainium2 NeuronCores.

Sharding: core = (batch b, query-quarter qb).  Each core computes the full
K/V chain for its batch (duplicated across the 4 cores sharing the batch;
~5 GFLOP) and full attention + o_proj for its own 512 query rows.  No
collectives.  All GEMMs in bf16 with fp32 PSUM accumulation; transposed
data layout end-to-end so no on-device transposes are needed (host passes
x^T per batch).

Shapes (hardcoded): x [2, 2048, 2048] f32, wq_d/wkv_d [2048, 256],
wq_u/wk_u [256, 1024], wv_u [256, 2048], wo [2048, 2048]; 16 heads,
head_dim 128, q/k head dim 64 (RoPE), latent 256.
"""

import numpy as np
import ml_dtypes

import concourse.bass as bass
import concourse.mybir as mybir
import concourse.tile as tile
from concourse import bacc
from concourse.bass_utils import run_bass_kernel_spmd

DIM = 2048
HEADS = 16
HEAD_DIM = 128
HALF = 64
LATENT = 256
B = 2
S = 2048
NQ = 512  # queries per core
P = 128
DC = DIM // P     # 16 contraction chunks over model dim
LC = LATENT // P  # 2 chunks over latent
SC = S // 512     # 4 free chunks of 512 over sequence
KC = S // P       # 16 key chunks of 128

BF16 = mybir.dt.bfloat16
F32 = mybir.dt.float32
NPBF = ml_dtypes.bfloat16

TRACE = False
LAST_RESULTS = None
_CACHE = {}


def _emit(nc, tc, t):
    mult = mybir.AluOpType.mult
    add = mybir.AluOpType.add
    Exp = mybir.ActivationFunctionType.Exp

    open_pools = {}

    def popen(name, bufs=1, space="SBUF"):
        cm = tc.tile_pool(name=name, bufs=bufs, space=space)
        pool = cm.__enter__()
        open_pools[name] = cm
        return pool

    def pclose(*names):
        for n in names:
            open_pools.pop(n).__exit__(None, None, None)

    smallp = popen("smallp", bufs=2)
    aop = popen("aop")
    wup = popen("wup")
    tabs = popen("tabs")
    latp = popen("latp")
    psA = popen("psA", bufs=4, space="PSUM")
    xtp = popen("xtp")
    wdown = popen("wdown")

    # ---------------- input loads ----------------
    xts = []
    xt_r = t["xt"].rearrange("(o p) s -> p o s", p=P)
    for i in range(4):
        xi = xtp.tile([P, 4, S], BF16, tag=f"xt{i}", name=f"xt{i}")
        nc.sync.dma_start(xi, xt_r[:, 4 * i : 4 * i + 4, :])
        xts.append(xi)
    qxt = xtp.tile([P, DC, NQ], BF16, tag="qxt")
    nc.sync.dma_start(qxt, t["qxt"].rearrange("(o p) q -> p o q", p=P))
    wkvd = wdown.tile([P, DC, LATENT], BF16, tag="wkvd")
    nc.sync.dma_start(wkvd, t["wkvd"].rearrange("(o p) l -> p o l", p=P))
    wqd = wdown.tile([P, DC, LATENT], BF16, tag="wqd")
    nc.sync.dma_start(wqd, t["wqd"].rearrange("(o p) l -> p o l", p=P))
    wku = wup.tile([P, LC, HEADS * HALF], BF16, tag="wku")
    nc.sync.dma_start(wku, t["wku"].rearrange("(o p) h -> p o h", p=P))
    wqu = wup.tile([P, LC, HEADS * HALF], BF16, tag="wqu")
    nc.sync.dma_start(wqu, t["wqu"].rearrange("(o p) h -> p o h", p=P))
    wkup = wup.tile([P, LC, HEADS * HALF], BF16, tag="wkup")
    nc.sync.dma_start(wkup, t["wkup"].rearrange("(o p) h -> p o h", p=P))
    wqup = wup.tile([P, LC, HEADS * HALF], BF16, tag="wqup")
    nc.sync.dma_start(wqup, t["wqup"].rearrange("(o p) h -> p o h", p=P))
    wvu = wup.tile([P, LC, DIM], BF16, tag="wvu")
    nc.sync.dma_start(wvu, t["wvu"].rearrange("(o p) f -> p o f", p=P))
    cosr = tabs.tile([P, S], BF16, tag="cosr")
    nc.sync.dma_start(cosr, t["cosr"])
    sinr = tabs.tile([P, S], BF16, tag="sinr")
    nc.sync.dma_start(sinr, t["sinr"])
    cosq = tabs.tile([P, NQ], BF16, tag="cosq")
    nc.sync.dma_start(cosq, t["cosq"])
    sinq = tabs.tile([P, NQ], BF16, tag="sinq")
    nc.sync.dma_start(sinq, t["sinq"])
    # ---------------- phase 1: latents (transposed) ----------------
    latkv = [latp.tile([P, S], BF16, tag=f"latkv{l}", name=f"latkv{l}") for l in range(LC)]
    latq = [latp.tile([P, NQ], BF16, tag=f"latq{l}", name=f"latq{l}") for l in range(LC)]
    for l in range(LC):
        for sc in range(SC):
            ps = psA.tile([P, 512], F32, tag="psA")
            for dc in range(DC):
                nc.tensor.matmul(
                    ps,
                    wkvd[:, dc, l * P : (l + 1) * P],
                    xts[dc // 4][:, dc % 4, sc * 512 : (sc + 1) * 512],
                    start=(dc == 0),
                    stop=(dc == DC - 1),
                )
            nc.scalar.copy(latkv[l][:, sc * 512 : (sc + 1) * 512], ps)
        ps = psA.tile([P, 512], F32, tag="psA")
        for dc in range(DC):
            nc.tensor.matmul(
                ps,
                wqd[:, dc, l * P : (l + 1) * P],
                qxt[:, dc, :],
                start=(dc == 0),
                stop=(dc == DC - 1),
            )
        nc.scalar.copy(latq[l], ps)

    pclose("wdown", "xtp")

    # ---------------- phase 2: K^T, Q^T (RoPE), V ----------------
    kqp = popen("kqp")
    vp = popen("vp")
    ropet = popen("ropet", bufs=1)

    kSb = [kqp.tile([P, S], BF16, tag=f"k{j}", name=f"k{j}") for j in range(8)]
    qSb = [kqp.tile([P, NQ], BF16, tag=f"q{j}", name=f"q{j}") for j in range(8)]
    vSb = [vp.tile([P, DIM], BF16, tag=f"v{sc}", name=f"v{sc}") for sc in range(KC)]

    def rope(dst, raw, shift, cos_t, sin_t):
        # dst = raw * cos + rotate_half = raw*cos + shift*sin, all same-partition
        tmpc = ropet.tile([P, raw.shape[-1]], BF16, tag="tmpc", name="tmpc")
        tmps = ropet.tile([P, raw.shape[-1]], BF16, tag="tmps", name="tmps")
        nc.vector.tensor_tensor(tmpc, raw, cos_t, mult)
        nc.vector.tensor_tensor(tmps, shift, sin_t, mult)
        nc.vector.tensor_tensor(dst, tmpc, tmps, add)

    for j in range(8):
        kraw = ropet.tile([P, S], BF16, tag="kraw", name="kraw")
        kshift = ropet.tile([P, S], BF16, tag="kshift", name="kshift")
        for w_t, dst in ((wku, kraw), (wkup, kshift)):
            for sc in range(SC):
                ps = psA.tile([P, 512], F32, tag="psA")
                for l in range(LC):
                    nc.tensor.matmul(
                        ps,
                        w_t[:, l, j * P : (j + 1) * P],
                        latkv[l][:, sc * 512 : (sc + 1) * 512],
                        start=(l == 0),
                        stop=(l == LC - 1),
                    )
                nc.scalar.copy(dst[:, sc * 512 : (sc + 1) * 512], ps)
        rope(kSb[j], kraw, kshift, cosr, sinr)

        qraw = ropet.tile([P, NQ], BF16, tag="qraw", name="qraw")
        qshift = ropet.tile([P, NQ], BF16, tag="qshift", name="qshift")
        for w_t, dst in ((wqu, qraw), (wqup, qshift)):
            ps = psA.tile([P, 512], F32, tag="psA")
            for l in range(LC):
                nc.tensor.matmul(
                    ps,
                    w_t[:, l, j * P : (j + 1) * P],
                    latq[l],
                    start=(l == 0),
                    stop=(l == LC - 1),
                )
            nc.scalar.copy(dst, ps)
        rope(qSb[j], qraw, qshift, cosq, sinq)

    for sc in range(KC):
        for fc in range(4):
            ps = psA.tile([P, 512], F32, tag="psA")
            for l in range(LC):
                nc.tensor.matmul(
                    ps,
                    latkv[l][:, sc * P : (sc + 1) * P],
                    wvu[:, l, fc * 512 : (fc + 1) * 512],
                    start=(l == 0),
                    stop=(l == LC - 1),
                )
            nc.vector.tensor_copy(vSb[sc][:, fc * 512 : (fc + 1) * 512], ps)

    pclose("ropet")
    pclose("psA")

    # ---------------- phase 3: attention, by head pair ----------------
    psS = popen("psS", bufs=2, space="PSUM")
    psV = popen("psV", bufs=2, space="PSUM")
    psD = popen("psD", bufs=1, space="PSUM")
    ptp = popen("ptp", bufs=8)
    ones_t = smallp.tile([P, 1], BF16, tag="ones", name="ones")
    nc.any.memset(ones_t, 1.0)

    aoSb = [aop.tile([P, NQ], BF16, tag=f"ao{h}", name=f"ao{h}") for h in range(HEADS)]
    for j in range(8):
        pts = []
        dens = [psD.tile([1, NQ], F32, tag=f"den{u}", name=f"den{u}") for u in range(2)]
        for kc in range(KC):
            stp = psS.tile([P, 2, 512], F32, tag="st")
            nc.tensor.matmul(
                stp[:, 0, :],
                kSb[j][0:HALF, kc * P : (kc + 1) * P],
                qSb[j][0:HALF, :],
                start=True,
                stop=True,
                tile_position=(0, 0),
            )
            nc.tensor.matmul(
                stp[:, 1, :],
                kSb[j][HALF:P, kc * P : (kc + 1) * P],
                qSb[j][HALF:P, :],
                start=True,
                stop=True,
                tile_position=(64, 0),
            )
            pt = ptp.tile([P, 2, 512], BF16, tag="pt")
            nc.scalar.activation(pt, stp, Exp, scale=0.125)
            pts.append(pt)
            nc.tensor.matmul(
                dens[0], ones_t, pt[:, 0, :], start=(kc == 0), stop=(kc == KC - 1)
            )
            nc.tensor.matmul(
                dens[1], ones_t, pt[:, 1, :], start=(kc == 0), stop=(kc == KC - 1)
            )
        for u in range(2):
            h = 2 * j + u
            av = psV.tile([P, NQ], F32, tag="av")
            for kc in range(KC):
                nc.tensor.matmul(
                    av,
                    vSb[kc][:, h * HEAD_DIM : (h + 1) * HEAD_DIM],
                    pts[kc][:, u, :],
                    start=(kc == 0),
                    stop=(kc == KC - 1),
                )
            rec = smallp.tile([1, NQ], F32, tag="rec", name="rec")
            nc.vector.reciprocal(rec, dens[u])
            recB = smallp.tile([P, NQ], F32, tag="recB", name="recB")
            nc.gpsimd.partition_broadcast(recB, rec)
            nc.vector.tensor_tensor(aoSb[h], av, recB, mult)

    pclose("ptp")
    pclose("psD", "psV", "psS")
    pclose("vp", "kqp")
    pclose("latp", "tabs", "wup")

    # ---------------- phase 4: o_proj ----------------
    wop = popen("wop")
    outp = popen("outp", bufs=2)
    psO = popen("psO", bufs=2, space="PSUM")

    wo_r = t["wo"].rearrange("(o p) f -> p o f", p=P)
    wos = []
    for i in range(4):
        wi = wop.tile([P, 4, DIM], BF16, tag=f"wo{i}", name=f"wo{i}")
        nc.sync.dma_start(wi, wo_r[:, 4 * i : 4 * i + 4, :])
        wos.append(wi)
    for qc in range(4):
        outs = outp.tile([P, DIM], F32, tag="outs")
        for ofc in range(4):
            ps = psO.tile([P, 512], F32, tag="psO")
            for fc in range(HEADS):
                nc.tensor.matmul(
                    ps,
                    aoSb[fc][:, qc * P : (qc + 1) * P],
                    wos[fc // 4][:, fc % 4, ofc * 512 : (ofc + 1) * 512],
                    start=(fc == 0),
                    stop=(fc == HEADS - 1),
                )
            if ofc % 2 == 0:
                nc.scalar.copy(outs[:, ofc * 512 : (ofc + 1) * 512], ps)
            else:
                nc.vector.tensor_copy(outs[:, ofc * 512 : (ofc + 1) * 512], ps)
        nc.sync.dma_start(t["out"][qc * P : (qc + 1) * P, :], outs)

    pclose("psO")
    pclose("outp", "wop")
    pclose("aop", "smallp")


def _build():
    if "nc" in _CACHE:
        return _CACHE["nc"]
    nc = bacc.Bacc(
        "TRN2", target_bir_lowering=False, debug=False, num_devices=8
    )
    t = {}
    for name, shape in [
        ("xt", [DIM, S]),
        ("qxt", [DIM, NQ]),
        ("wqd", [DIM, LATENT]),
        ("wkvd", [DIM, LATENT]),
        ("wqu", [LATENT, HEADS * HALF]),
        ("wku", [LATENT, HEADS * HALF]),
        ("wkup", [LATENT, HEADS * HALF]),
        ("wqup", [LATENT, HEADS * HALF]),
        ("wvu", [LATENT, DIM]),
        ("wo", [DIM, DIM]),
        ("cosr", [P, S]),
        ("sinr", [P, S]),
        ("cosq", [P, NQ]),
        ("sinq", [P, NQ]),
    ]:
        t[name] = nc.dram_tensor(name, shape, BF16, kind="ExternalInput").ap()
    t["out"] = nc.dram_tensor("out", [NQ, DIM], F32, kind="ExternalOutput").ap()

    with tile.TileContext(nc, pool_alloc_mode="queue") as tc:
        _emit(nc, tc, t)
    nc.compile()
    _CACHE["nc"] = nc
    return nc


def _host_tables():
    inv = 1.0 / (10000.0 ** (np.arange(0, HALF, 2, dtype=np.float64) / HALF))
    tpos = np.arange(S, dtype=np.float64)
    fr = np.outer(tpos, inv)  # [S, 32]
    cosv = np.cos(fr).T.astype(np.float32)  # [32, S]
    sinv = np.sin(fr).T.astype(np.float32)
    cosr = np.tile(cosv, (4, 1))
    sinr = np.tile(sinv, (4, 1))
    return cosr.astype(NPBF), sinr.astype(NPBF)


def _perm_rot(w):
    # columns h*64+j -> rotate_half: out[:, j<32] = -w[:, j+32]; out[:, j>=32] = w[:, j-32]
    w = np.asarray(w, dtype=np.float32).reshape(LATENT, HEADS, HALF)
    out = np.concatenate([-w[:, :, 32:], w[:, :, :32]], axis=2)
    return out.reshape(LATENT, HEADS * HALF)


def prep_maps(x, wq_d, wkv_d, wq_u, wk_u, wv_u, wo):
    def bf(a):
        return np.ascontiguousarray(np.asarray(a, dtype=np.float32)).astype(NPBF)

    x = np.asarray(x, dtype=np.float32)
    xT = [bf(x[b].T) for b in range(B)]
    w = {
        "wqd": bf(wq_d),
        "wkvd": bf(wkv_d),
        "wqu": bf(wq_u),
        "wku": bf(wk_u),
        "wkup": bf(_perm_rot(wk_u)),
        "wqup": bf(_perm_rot(wq_u)),
        "wvu": bf(wv_u),
        "wo": bf(wo),
    }
    cosr, sinr = _host_tables()

    in_maps = []
    for core in range(8):
        b, qb = core // 4, core % 4
        sl = slice(qb * NQ, (qb + 1) * NQ)
        m = dict(w)
        m["xt"] = xT[b]
        m["qxt"] = np.ascontiguousarray(xT[b][:, sl])
        m["cosr"] = cosr
        m["sinr"] = sinr
        m["cosq"] = np.ascontiguousarray(cosr[:, sl])
        m["sinq"] = np.ascontiguousarray(sinr[:, sl])
        in_maps.append(m)
    return in_maps


def kernel(x, wq_d, wkv_d, wq_u, wk_u, wv_u, wo):
    global LAST_RESULTS
    nc = _build()
    in_maps = prep_maps(x, wq_d, wkv_d, wq_u, wk_u, wv_u, wo)

    res = run_bass_kernel_spmd(
        nc, in_maps, core_ids=list(range(8)), trace=TRACE
    )
    LAST_RESULTS = res
    full = np.empty((B, S, DIM), np.float32)
    for core in range(8):
        b, qb = core // 4, core % 4
        full[b, qb * NQ : (qb + 1) * NQ, :] = res.results[core]["out"]
    return full
